# revision 6
# baseline (speedup 1.0000x reference)
"""Trainium2 Bass kernel for nn_BoundaryDiceLoss_82171314307268.

Sharding: pure data-parallel over 8 cores; core c handles sample c//2,
D-half c%2. Host preps per-core slabs in [H=128(partitions), D-slots,
w] layout (64 owned D slices + 3 halo each side, D edge-replicated):
  dif  [128, 64*128]  bf16  packed w: output[s,1]-output[s,0]
  tgt  [128, 64*128]  bf16  target mask {0,1}
  vst  [128, 70*133]  fp8e4 padded w (cols [2,130) data, col1/130
        edge-replicated, cols 0,131,132 zero):
        v = 1 + (dif>0) + 7*target  in {1,2,8,9}  (combined state)

Per-core algorithm (all conv work on the PE in fp8 DoubleRow pairs —
two moving fields + two weight planes per matmul at 0.5 cycles/row):
  Boundary:  D = c_v - 6v  (6-neighbor sum minus 6*center; carry-free
    for {1,2,8,9}, so D != 0 iff some neighbor differs in either mask).
    5 shifted fields of v as 3 DR matmuls/chunk; E = |D| via ACT Abs
    -> e3 fp8.  WP=133 makes every k-pair stride = 0 mod 4 (the HW
    requirement for the DR k-tile stride at 512-wide psum).
  Region:    r = conv3d(E, ball radius 2), exact 33-point ball as 13
    H-band fields of e3 in 7 DR matmuls/chunk; m = r > 0.5 via DVE
    tensor_scalar is_gt reading PSUM directly.
  probs p = sigmoid(dif) (ACT), products split DVE/Pool:
    b = t*m, q1 = p*b (= p*t*m), qA = p*m; sums via DVE tensor_scalar
    4x-mode accumulate -> acc cols; host: S1 = sum q1, S2 = sum qA +
    sum b, dice = (2 S1 + eps)/(S2 + eps).
  nonempty check on host (boundary of either mask nonempty).
"""
import sys

sys.path.insert(0, "/opt/trn_rl_repo")

import numpy as np
import ml_dtypes

import concourse.bass as bass
import concourse.bacc as bacc
import concourse.tile as tile
import concourse.mybir as mybir
from concourse.ap import AP
from concourse.bass_utils import run_bass_kernel_spmd

f32 = mybir.dt.float32
bf16 = mybir.dt.bfloat16
f8 = mybir.dt.float8e4
Alu = mybir.AluOpType
Act = mybir.ActivationFunctionType
DR = mybir.MatmulPerfMode.DoubleRow

P = 128          # H on partitions
W = 128
OWN = 64         # owned D slices per core
HALO = 3
DEXT = OWN + 2 * HALO          # 70 slab D-slots
WP = 133                       # padded w stride; data cols [2, 130)
B = 4
EPS = 1e-05
CH = 4                         # slots per chunk (512 free / psum bank)
NEC = 17                       # E chunks (slots 1..68)
NDC = 16                       # dilation chunks (owned slots 3..66)
NMAT = 7                       # weight-pair matrices
ACC_COLS = 16

# DVE-owned qA chunks (rest on Pool) - load balance Pool vs DVE
QA_DVE = {1, 4, 7, 10, 13, 15}


def _band(offsets, rep_edges=False):
    m = np.zeros((P, P), np.float32)
    for o in offsets:
        for i in range(P):
            j = i + o
            if 0 <= j < P:
                m[j, i] += 1.0
            elif rep_edges:
                m[min(max(j, 0), P - 1), i] += 1.0
    return m


def _mats_all():
    """7 DoubleRow lhsT pairs, each [128, 2, 128] -> [128, 7*256]."""
    ident = np.eye(P, dtype=np.float32)
    t3 = _band([-1, 0, 1])
    t5 = _band([-2, -1, 0, 1, 2])
    a1 = _band([-1, 1], rep_edges=True)
    m_b = a1 - 6.0 * ident
    zero = np.zeros((P, P), np.float32)
    pairs = [
        (ident, ident),   # 0 ME_idid : PE1, PE2
        (m_b, zero),      # 1 ME_mb0  : PE3
        (ident, t3),      # 2 MD_t1t3 : PD1, PD5
        (t3, t3),         # 3 MD_t3t3 : PD2, PD4
        (t3, t5),         # 4 MD_t3t5 : PD3
        (ident, ident),   # 5 MD_t1t1 : PD6
        (t3, zero),       # 6 MD_t30  : PD7
    ]
    out = np.zeros((P, NMAT, 2, P), np.float32)
    for i, (w0, w1) in enumerate(pairs):
        out[:, i, 0, :] = w0
        out[:, i, 1, :] = w1
    return out.reshape(P, NMAT * 2 * P)


# (kind, k0 field (dz, dx)); k1 field = k0 + (dk_slots, dk_cols) where
# delta = dk_slots*WP + dk_cols must be >= 0 and = 0 mod 4.
# E phase fields on v (pair delta in elements):
E_PAIRS = [
    (0, (-1, 0), WP - 1),   # k0 id @ v[z-1], k1 id @ v[w-1]
    (0, (0, 1), WP - 1),    # k0 id @ v[w+1], k1 id @ v[z+1]
    (1, (0, 0), 4),         # k0 m_b @ v,     k1 zero (pad)
]
# dilation fields on e3:
D_PAIRS = [
    (2, (-2, 0), WP - 1),   # T1@(-2,0)  + T3@(-1,-1)
    (3, (-1, 0), WP - 1),   # T3@(-1,0)  + T3@(0,-1)
    (4, (-1, 1), WP - 1),   # T3@(-1,1)  + T5@(0,0)
    (3, (0, 1), WP - 1),    # T3@(0,1)   + T3@(1,0)
    (2, (0, -2), WP + 3),   # T1@(0,-2)  + T3@(1,1)
    (5, (0, 2), 2 * WP - 2),  # T1@(0,2)  + T1@(2,0)
    (6, (1, -1), 4),        # T3@(1,-1)  + zero (pad)
]


def _dr_rhs(slab3, s0, dz, dx, delta):
    """rhs AP [p][k=2, stride delta][s=CH][w=128] for field (dz, dx)."""
    base = slab3[:, s0 + dz:s0 + dz + CH, 2 + dx:130 + dx]
    bp = [list(d) for d in base.ap]
    return AP(base.tensor, base.offset, [bp[0], [delta, 2], bp[1], bp[2]])


def _build_program():
    nc = bacc.Bacc("TRN2", target_bir_lowering=False, debug=False,
                   num_devices=8)
    d_dif = nc.dram_tensor("dif", [P, OWN * W], bf16, kind="ExternalInput")
    d_tgt = nc.dram_tensor("tgt", [P, OWN * W], bf16, kind="ExternalInput")
    d_v = nc.dram_tensor("vst", [P, DEXT * WP], f8, kind="ExternalInput")
    d_mats = nc.dram_tensor("mats", [P, NMAT * 2 * P], f8,
                            kind="ExternalInput")
    d_psums = nc.dram_tensor("psums", [P, ACC_COLS], f32,
                             kind="ExternalOutput")

    with tile.TileContext(nc) as tc:
        with tc.tile_pool(name="consts", bufs=1) as cp, \
             tc.tile_pool(name="slabs", bufs=1) as sp, \
             tc.tile_pool(name="ps_e", bufs=4, space="PSUM") as ps_e, \
             tc.tile_pool(name="ps_d", bufs=4, space="PSUM") as ps_d:

            matst = cp.tile([P, NMAT * 2 * P], f8, tag="mats", name="mats")
            nc.sync.dma_start(matst[:], d_mats[:])
            mats4 = matst[:].rearrange("p (m k j) -> p m k j", m=NMAT, k=2)

            def slab(name_, cols, dtype, slots):
                t = sp.tile([P, slots * cols], dtype, tag=name_, name=name_)
                return t.rearrange("p (s w) -> p s w", w=cols)

            v3 = slab("v", WP, f8, DEXT)
            e3 = slab("e", WP, f8, DEXT)
            dif3 = slab("dif", W, bf16, OWN)  # raw logit diff, owned
            p3 = slab("p", W, bf16, OWN)     # sigmoid probs, owned
            t3f = slab("t", W, bf16, OWN)    # target mask, owned
            m3 = slab("m", W, bf16, OWN)     # region mask
            b3 = slab("b", W, bf16, OWN)     # t*m
            q1f = slab("q1", W, bf16, OWN)   # p*b
            qAf = slab("qA", W, bf16, OWN)   # p*m
            acc = sp.tile([P, ACC_COLS], f32, tag="acc", name="acc")

            # zero E pads: slots 0,69 and w-pad cols (never written again)
            nc.vector.memset(e3[:, 0, :], 0.0)
            nc.vector.memset(e3[:, 69, :], 0.0)
            nc.vector.memset(e3[:, 1:69, 0:2], 0.0)
            nc.vector.memset(e3[:, 1:69, 130:133], 0.0)

            # ---- input DMAs: v first (gates E), dif/tgt on gpsimd q ----
            vsplit = [(0, 18), (18, 18), (36, 18), (54, 16)]
            for s0, ns in vsplit:
                nc.sync.dma_start(
                    v3[:, s0:s0 + ns, :].rearrange("p s w -> p (s w)"),
                    d_v[:, s0 * WP:(s0 + ns) * WP])
            for k in range(4):
                fs = slice(k * 16 * W, (k + 1) * 16 * W)
                ksl = slice(k * 16, (k + 1) * 16)
                nc.gpsimd.dma_start(
                    dif3[:, ksl, :].rearrange("p s w -> p (s w)"),
                    d_dif[:, fs])
                nc.gpsimd.dma_start(
                    t3f[:, ksl, :].rearrange("p s w -> p (s w)"),
                    d_tgt[:, fs])

            # E-chunk starts: slots 1..68 in 4-slot chunks (17 chunks)
            # dil-chunk j: owned slots [4j, 4j+4) = slab slots 3+4j..
            # Interleave: E batches of 4 chunks; dil batch b after E
            # batch b+1 complete.
            ebatches = [list(range(4 * i, min(4 * i + 4, NEC)))
                        for i in range((NEC + 3) // 4)]   # 5 batches
            dbatches = [list(range(4 * i, 4 * i + 4)) for i in range(4)]

            eps_tiles = {}
            dps_tiles = {}

            def emit_E_batch(chunks):
                # kind-major: mat0 fields PE1,PE2 for all chunks, then mat1
                for mi, pairlist in ((0, E_PAIRS[0:2]), (1, E_PAIRS[2:3])):
                    for c in chunks:
                        s0 = 1 + 4 * c
                        if c not in eps_tiles:
                            eps_tiles[c] = ps_e.tile(
                                [P, CH * W], f32, tag="eps", name=f"eps{c}")
                        pt = eps_tiles[c]
                        pv = pt[:].rearrange("p (s w) -> p s w", w=W)
                        for pi, (kind, (dz, dx), delta) in enumerate(
                                pairlist):
                            first = (mi == 0 and pi == 0)
                            last = (mi == 1)
                            nc.tensor.matmul(
                                pv[:], mats4[:, kind, :, :],
                                _dr_rhs(v3, s0, dz, dx, delta),
                                start=first, stop=last, perf_mode=DR,
                                skip_group_check=True)
                # Abs -> e3 (ACT), frees psum
                for c in chunks:
                    s0 = 1 + 4 * c
                    pt = eps_tiles.pop(c)
                    nc.scalar.activation(
                        e3[:, s0:s0 + CH, 2:130],
                        pt[:].rearrange("p (s w) -> p s w", w=W), Act.Abs)

            def emit_D_batch(chunks):
                for mi_idx, (mi, plist) in enumerate((
                        (2, [D_PAIRS[0], D_PAIRS[4]]),
                        (3, [D_PAIRS[1], D_PAIRS[3]]),
                        (4, [D_PAIRS[2]]),
                        (5, [D_PAIRS[5]]),
                        (6, [D_PAIRS[6]]))):
                    for c in chunks:
                        s0 = 3 + 4 * c
                        if c not in dps_tiles:
                            dps_tiles[c] = ps_d.tile(
                                [P, CH * W], f32, tag="dps", name=f"dps{c}")
                        pt = dps_tiles[c]
                        pv = pt[:].rearrange("p (s w) -> p s w", w=W)
                        for pi, (kind, (dz, dx), delta) in enumerate(plist):
                            first = (mi_idx == 0 and pi == 0)
                            last = (mi_idx == 4)
                            nc.tensor.matmul(
                                pv[:], mats4[:, kind, :, :],
                                _dr_rhs(e3, s0, dz, dx, delta),
                                start=first, stop=last, perf_mode=DR,
                                skip_group_check=True)
                # consumers per chunk
                for c in chunks:
                    jj = slice(4 * c, 4 * c + 4)
                    pt = dps_tiles.pop(c)
                    # m = (r > 0.5) on DVE from PSUM
                    nc.vector.tensor_scalar(
                        m3[:, jj, :],
                        pt[:].rearrange("p (s w) -> p s w", w=W),
                        0.5, None, op0=Alu.is_gt)
                    nc.gpsimd.tensor_tensor(
                        b3[:, jj, :], t3f[:, jj, :], m3[:, jj, :],
                        op=Alu.mult)
                    nc.gpsimd.tensor_tensor(
                        q1f[:, jj, :], p3[:, jj, :], b3[:, jj, :],
                        op=Alu.mult)
                    eng = nc.vector if c in QA_DVE else nc.gpsimd
                    eng.tensor_tensor(
                        qAf[:, jj, :], p3[:, jj, :], m3[:, jj, :],
                        op=Alu.mult)

            def emit_sums(g):
                # per 4-chunk group g: accumulate 16 slots
                jj = slice(16 * g, 16 * g + 16)
                for i, srcf in enumerate((b3, q1f, qAf)):
                    nc.vector.tensor_scalar(
                        srcf[:, jj, :], srcf[:, jj, :], 1.0, 0.0,
                        op0=Alu.mult, op1=Alu.add,
                        accum_out=acc[:, 3 * g + i:3 * g + i + 1])

            # probs quarters inserted into ACT stream between E batches
            def emit_probs(k):
                ksl = slice(k * 16, (k + 1) * 16)
                nc.scalar.activation(p3[:, ksl, :], dif3[:, ksl, :],
                                     Act.Sigmoid)

            emit_E_batch(ebatches[0])
            emit_probs(0)
            emit_E_batch(ebatches[1])
            emit_probs(1)
            emit_D_batch(dbatches[0])
            emit_E_batch(ebatches[2])
            emit_probs(2)
            emit_D_batch(dbatches[1])
            emit_sums(0)
            emit_E_batch(ebatches[3])
            emit_probs(3)
            emit_D_batch(dbatches[2])
            emit_sums(1)
            emit_E_batch(ebatches[4])
            emit_D_batch(dbatches[3])
            emit_sums(2)
            emit_sums(3)

            nc.sync.dma_start(d_psums[:], acc[:])

    nc.compile()
    return nc


_CACHE = {}
TRACE = False
_LAST = {"exec_time_ns": None, "results": None}


def _get_program():
    if "nc" not in _CACHE:
        _CACHE["nc"] = _build_program()
    return _CACHE["nc"]


def last_exec_time_ns():
    return _LAST["exec_time_ns"]


def kernel(output, target):
    output = np.asarray(output, dtype=np.float32)
    target = np.asarray(target, dtype=np.float32)
    nc = _get_program()

    dif = output[:, 1] - output[:, 0]                  # [B, D, H, W]
    vfull = (dif > 0).astype(np.float32) + 7.0 * target[:, 0] + 1.0
    vpad = np.pad(vfull, ((0, 0), (HALO, HALO), (0, 0), (0, 0)),
                  mode="edge")
    vp = np.zeros(vpad.shape[:3] + (WP,), np.float32)
    vp[..., 2:130] = vpad
    vp[..., 1] = vpad[..., 0]
    vp[..., 130] = vpad[..., 127]
    vp = vp.astype(ml_dtypes.float8_e4m3)
    dif16 = dif.astype(ml_dtypes.bfloat16)
    tgt16 = target[:, 0].astype(ml_dtypes.bfloat16)

    mats = _mats_all().astype(ml_dtypes.float8_e4m3)
    in_maps = []
    for c in range(8):
        s, h = c // 2, c % 2
        d0 = 0 if h == 0 else OWN
        vsl = np.ascontiguousarray(
            vp[s][d0:d0 + DEXT].transpose(1, 0, 2)).reshape(P, DEXT * WP)
        dsl = np.ascontiguousarray(
            dif16[s][d0:d0 + OWN].transpose(1, 0, 2)).reshape(P, OWN * W)
        tsl = np.ascontiguousarray(
            tgt16[s][d0:d0 + OWN].transpose(1, 0, 2)).reshape(P, OWN * W)
        in_maps.append({"dif": dsl, "vst": vsl, "tgt": tsl, "mats": mats})

    res = run_bass_kernel_spmd(nc, in_maps, list(range(8)), trace=TRACE)
    _LAST["exec_time_ns"] = res.exec_time_ns
    _LAST["results"] = res
    # nonempty <=> boundary set of either mask nonempty
    tmask = target[:, 0] > 0.5
    pmask = dif > 0
    nonempty = np.zeros(B, bool)
    for s in range(B):
        for msk in (tmask[s], pmask[s]):
            for ax in range(3):
                if nonempty[s]:
                    break
                nonempty[s] |= bool(np.any(np.diff(msk, axis=ax)))
    parts = np.zeros((B, 3), np.float64)   # [sum_b, sum_q1, sum_qA]
    for c in range(8):
        ps = res.results[c]["psums"].astype(np.float64)  # [128, 16]
        for g in range(4):
            for i in range(3):
                parts[c // 2, i] += ps[:, 3 * g + i].sum()
    s_b, s_q1, s_qA = parts.T
    s_ptm = s_q1
    s_card = s_qA + s_b
    dice = (2.0 * s_ptm + EPS) / (s_card + EPS)
    per_sample = np.where(nonempty, 1.0 - dice, 0.0)
    return np.float32(per_sample.sum() / B)


# revision 8
# speedup vs baseline: 1.5796x; 1.5796x over previous
"""Trainium2 Bass kernel for nn_BoundaryDiceLoss_82171314307268.

Sharding: pure data-parallel over 8 cores; core c handles sample c//2,
D-half c%2. Host preps per-core slabs in [H=128(partitions), D-slots,
w] layout (64 owned D slices + 3 halo each side, D edge-replicated):
  dif  [128, 64*128]  bf16  packed w: output[s,1]-output[s,0]
  tgt  [128, 64*128]  bf16  target mask {0,1}
  vst  [128, 70*133]  fp8e4 padded w (cols [2,130) data, col1/130
        edge-replicated, cols 0,131,132 zero):
        v = 1 + (dif>0) + 7*target  in {1,2,8,9}  (combined state)

Per-core algorithm:
  Boundary:  D = c_v - 6v (carry-free for {1,2,8,9}: D != 0 iff some
    6-neighbor differs in either mask). 5 shifted v-fields as 3 fp8
    DoubleRow matmuls/chunk (2 fields per mm); E = |D| via ACT Abs ->
    fp8 slab inside one hand-laid mega tile.
  Composites on the DMA engines (gpsimd SWDGE accum_op=add, CCE):
    s3z = E[z-1]+E[z+1]; F = s3z + E; c4b = E[z-2]+E[z+2]
    (fp8 add rounding keeps zero/positive exactly - only positivity
    matters downstream). Built in 2 z-strips pipelined with E.
  Region: r = conv3d(E, ball radius 2) exactly, via 7 fields
    {E:T5, s3z:T3, F[w-1]:T3, F[w+1]:T3, E[w-2]:T1, E[w+2]:T1, c4b:T1}
    in 4 DR matmuls/chunk. Mega-tile slab offsets are chosen so every
    DR k-pair stride is positive and = 0 mod 4 (HW requirement).
  Products read PSUM directly: per chunk two DVE STTs
    (r > 0.5)*pt and (r > 0.5)*ps with accum_out (pt = p*t, ps = p+t
    precomputed; p = sigmoid(dif) on ACT). Host: dice from the sums;
    nonempty check on host.
"""
import sys

sys.path.insert(0, "/opt/trn_rl_repo")

import numpy as np
import ml_dtypes

import concourse.bass as bass
import concourse.bacc as bacc
import concourse.tile as tile
import concourse.mybir as mybir
from concourse.ap import AP
from concourse.bass_utils import run_bass_kernel_spmd

f32 = mybir.dt.float32
bf16 = mybir.dt.bfloat16
f8 = mybir.dt.float8e4
Alu = mybir.AluOpType
Act = mybir.ActivationFunctionType
DR = mybir.MatmulPerfMode.DoubleRow

P = 128          # H on partitions
W = 128
OWN = 64         # owned D slices per core
HALO = 3
DEXT = OWN + 2 * HALO          # 70 slab D-slots
WP = 133                       # padded w stride; data cols [2, 130)
B = 4
EPS = 1e-05
CH = 4                         # slots per chunk (512 free / psum bank)
NEC = 17                       # E chunks (slots 1..68)
NDC = 16                       # dilation chunks (owned slots 3..66)
NMAT = 6
SLAB = DEXT * WP               # 9310 elements
SLABP = 9312                   # padded slab stride inside mega tile
# mega-tile slab offsets; o_E = o_F + SLABP + 1 makes the F->E DR
# k-pair delta 9312 (positive, mod 4 = 0 - the HW stride requirement):
O_F = 0
O_E = SLABP + 1                # 9313
O_S = O_E + SLABP              # 18625
O_C = O_S + SLABP              # 27937
MEGAP = O_C + SLABP + 4        # 37253 rounded
ACC_COLS = 32

# E phase DR pairs on v: (mat, k0 field (dz,dx), delta)
E_PAIRS = [
    (0, (-1, 0), WP - 1),   # id @ v[z-1], id @ v[w-1]
    (0, (0, 1), WP - 1),    # id @ v[w+1], id @ v[z+1]
    (1, (0, 0), 4),         # m_b @ v,     zero (pad)
]
# dilation DR pairs on the mega tile: (mat, (slab_offset, dx), delta)
D_PAIRS = [
    (2, (O_F, 1), SLABP),   # T3@F[w+1] + T5@E      (O_F+3 -> O_E+2)
    (3, (O_S, 0), SLABP),   # T3@s3z    + T1@c4b    (O_S+2 -> O_C+2)
    (4, (O_E, -2), 4),      # T1@E[w-2] + T1@E[w+2]
    (5, (O_F, -1), 4),      # T3@F[w-1] + zero (pad)
]


def _band(offsets, rep_edges=False):
    m = np.zeros((P, P), np.float32)
    for o in offsets:
        for i in range(P):
            j = i + o
            if 0 <= j < P:
                m[j, i] += 1.0
            elif rep_edges:
                m[min(max(j, 0), P - 1), i] += 1.0
    return m


def _mats_all():
    ident = np.eye(P, dtype=np.float32)
    t3 = _band([-1, 0, 1])
    t5 = _band([-2, -1, 0, 1, 2])
    a1 = _band([-1, 1], rep_edges=True)
    m_b = a1 - 6.0 * ident
    zero = np.zeros((P, P), np.float32)
    pairs = [
        (ident, ident),   # 0 E: PE1, PE2
        (m_b, zero),      # 1 E: PE3
        (t3, t5),         # 2 D: F[w+1] + E
        (t3, ident),      # 3 D: s3z + c4b
        (ident, ident),   # 4 D: E[w-2] + E[w+2]
        (t3, zero),       # 5 D: F[w-1] (pad)
    ]
    out = np.zeros((P, NMAT, 2, P), np.float32)
    for i, (w0, w1) in enumerate(pairs):
        out[:, i, 0, :] = w0
        out[:, i, 1, :] = w1
    return out.reshape(P, NMAT * 2 * P)


def _dr_rhs_v(v3, s0, dz, dx, delta):
    base = v3[:, s0 + dz:s0 + dz + CH, 2 + dx:130 + dx]
    bp = [list(d) for d in base.ap]
    return AP(base.tensor, base.offset, [bp[0], [delta, 2], bp[1], bp[2]])


def _dr_rhs_mega(megaap, s0, slab_o, dx, delta):
    mp = [list(d) for d in megaap.ap]
    off = slab_o + s0 * WP + 2 + dx
    return AP(megaap.tensor, off,
              [mp[0], [delta, 2], [WP, CH], [1, W]])


def _build_program():
    nc = bacc.Bacc("TRN2", target_bir_lowering=False, debug=False,
                   num_devices=8)
    d_dif = nc.dram_tensor("dif", [P, OWN * W], bf16, kind="ExternalInput")
    d_tgt = nc.dram_tensor("tgt", [P, OWN * W], bf16, kind="ExternalInput")
    d_v = nc.dram_tensor("vst", [P, DEXT * WP], f8, kind="ExternalInput")
    d_mats = nc.dram_tensor("mats", [P, NMAT * 2 * P], f8,
                            kind="ExternalInput")
    d_psums = nc.dram_tensor("psums", [P, ACC_COLS], f32,
                             kind="ExternalOutput")

    with tile.TileContext(nc) as tc:
        with tc.tile_pool(name="consts", bufs=1) as cp, \
             tc.tile_pool(name="slabs", bufs=1) as sp, \
             tc.tile_pool(name="scr", bufs=4) as kp, \
             tc.tile_pool(name="ps_e", bufs=4, space="PSUM") as ps_e, \
             tc.tile_pool(name="ps_d", bufs=4, space="PSUM") as ps_d:

            matst = cp.tile([P, NMAT * 2 * P], f8, tag="mats", name="mats")
            nc.sync.dma_start(matst[:], d_mats[:])
            mats4 = matst[:].rearrange("p (m k j) -> p m k j", m=NMAT, k=2)

            mega = sp.tile([P, MEGAP], f8, tag="mega", name="mega")
            e3 = mega[:, O_E:O_E + SLAB].rearrange("p (s w) -> p s w", w=WP)
            v3t = sp.tile([P, SLAB], f8, tag="v", name="v")
            v3 = v3t.rearrange("p (s w) -> p s w", w=WP)

            def bslab(name_):
                t = sp.tile([P, OWN * W], bf16, tag=name_, name=name_)
                return t.rearrange("p (s w) -> p s w", w=W)

            dif3 = bslab("dif")
            p3 = bslab("p")
            t3f = bslab("t")
            pt3 = bslab("pt")
            ps3 = bslab("ps")
            acc = sp.tile([P, ACC_COLS], f32, tag="acc", name="acc")

            # zero E pads (slots 0,69 + w-pad cols); composites inherit
            # zeros from E via the shifted copies.
            nc.vector.memset(e3[:, 0, :], 0.0)
            nc.vector.memset(e3[:, 69, :], 0.0)
            nc.vector.memset(e3[:, 1:69, 0:2], 0.0)
            nc.vector.memset(e3[:, 1:69, 130:133], 0.0)

            # ---- input DMAs (SP queue; v first - it gates E) ----
            vsplit = [(0, 18), (18, 18), (36, 18), (54, 16)]
            for s0, ns in vsplit:
                nc.sync.dma_start(
                    v3[:, s0:s0 + ns, :].rearrange("p s w -> p (s w)"),
                    d_v[:, s0 * WP:(s0 + ns) * WP])
            for k in range(4):
                fs = slice(k * 16 * W, (k + 1) * 16 * W)
                ksl = slice(k * 16, (k + 1) * 16)
                nc.sync.dma_start(
                    dif3[:, ksl, :].rearrange("p s w -> p (s w)"),
                    d_dif[:, fs])
                nc.sync.dma_start(
                    t3f[:, ksl, :].rearrange("p s w -> p (s w)"),
                    d_tgt[:, fs])

            ebatches = [list(range(4 * i, min(4 * i + 4, NEC)))
                        for i in range((NEC + 3) // 4)]   # 5 batches
            dbatches = [list(range(4 * i, 4 * i + 4)) for i in range(4)]
            eps_tiles = {}
            dps_tiles = {}

            def emit_E_batch(chunks):
                for mi, pairlist in ((0, E_PAIRS[0:2]), (1, E_PAIRS[2:3])):
                    for c in chunks:
                        s0 = 1 + 4 * c
                        if c not in eps_tiles:
                            eps_tiles[c] = ps_e.tile(
                                [P, CH * W], f32, tag="eps", name=f"eps{c}")
                        pt = eps_tiles[c]
                        pv = pt[:].rearrange("p (s w) -> p s w", w=W)
                        for pi, (kind, (dz, dx), delta) in enumerate(
                                pairlist):
                            nc.tensor.matmul(
                                pv[:], mats4[:, kind, :, :],
                                _dr_rhs_v(v3, s0, dz, dx, delta),
                                start=(mi == 0 and pi == 0),
                                stop=(mi == 1), perf_mode=DR,
                                skip_group_check=True)
                for c in chunks:
                    s0 = 1 + 4 * c
                    pt = eps_tiles.pop(c)
                    nc.scalar.activation(
                        e3[:, s0:s0 + CH, 2:130],
                        pt[:].rearrange("p (s w) -> p s w", w=W), Act.Abs)

            megaraw = mega[:]

            def emit_strip(lo, hi):
                """Composites for slots [lo, hi) subset of [3, 67)."""
                import os as _os
                if _os.environ.get("NOSTRIP") == "1":
                    return
                def seg(off, a, b):
                    return megaraw[:, off + a * WP:off + b * WP]
                # s3z = E[z-1] + E[z+1]
                nc.gpsimd.dma_start(seg(O_S, lo, hi),
                                    seg(O_E, lo - 1, hi - 1))
                nc.gpsimd.dma_start(seg(O_S, lo, hi),
                                    seg(O_E, lo + 1, hi + 1),
                                    accum_op=Alu.add)
                # c4b = E[z-2] + E[z+2]
                nc.gpsimd.dma_start(seg(O_C, lo, hi),
                                    seg(O_E, lo - 2, hi - 2))
                nc.gpsimd.dma_start(seg(O_C, lo, hi),
                                    seg(O_E, lo + 2, hi + 2),
                                    accum_op=Alu.add)
                # F = s3z + E
                nc.gpsimd.dma_start(seg(O_F, lo, hi), seg(O_S, lo, hi))
                nc.gpsimd.dma_start(seg(O_F, lo, hi), seg(O_E, lo, hi),
                                    accum_op=Alu.add)

            def emit_D_batch(chunks):
                for mi_idx, (kind, (slab_o, dx), delta) in enumerate(
                        D_PAIRS):
                    for c in chunks:
                        s0 = 3 + 4 * c
                        if c not in dps_tiles:
                            dps_tiles[c] = ps_d.tile(
                                [P, CH * W], f32, tag="dps", name=f"dps{c}")
                        pt = dps_tiles[c]
                        pv = pt[:].rearrange("p (s w) -> p s w", w=W)
                        nc.tensor.matmul(
                            pv[:], mats4[:, kind, :, :],
                            _dr_rhs_mega(megaraw, s0, slab_o, dx, delta),
                            start=(mi_idx == 0), stop=(mi_idx == 3),
                            perf_mode=DR, skip_group_check=True)
                for c in chunks:
                    jj = slice(4 * c, 4 * c + 4)
                    pt = dps_tiles.pop(c)
                    pv = pt[:].rearrange("p (s w) -> p s w", w=W)
                    scr = kp.tile([P, CH * W], bf16, tag="scr")
                    sc3 = scr[:].rearrange("p (s w) -> p s w", w=W)
                    nc.vector.scalar_tensor_tensor(
                        sc3[:], pv[:], 0.5, pt3[:, jj, :], op0=Alu.is_gt,
                        op1=Alu.mult, accum_out=acc[:, 2 * c:2 * c + 1])
                    nc.vector.scalar_tensor_tensor(
                        sc3[:], pv[:], 0.5, ps3[:, jj, :], op0=Alu.is_gt,
                        op1=Alu.mult,
                        accum_out=acc[:, 2 * c + 1:2 * c + 2])

            def emit_probs(k):
                ksl = slice(k * 16, (k + 1) * 16)
                nc.scalar.activation(p3[:, ksl, :], dif3[:, ksl, :],
                                     Act.Sigmoid)
                nc.vector.tensor_tensor(pt3[:, ksl, :], p3[:, ksl, :],
                                        t3f[:, ksl, :], op=Alu.mult)
                nc.vector.tensor_tensor(ps3[:, ksl, :], p3[:, ksl, :],
                                        t3f[:, ksl, :], op=Alu.add)

            emit_E_batch(ebatches[0])
            emit_probs(0)
            emit_E_batch(ebatches[1])
            emit_probs(1)
            emit_E_batch(ebatches[2])
            emit_strip(3, 35)          # composites for slots [3,35)
            emit_probs(2)
            emit_E_batch(ebatches[3])
            emit_probs(3)
            emit_E_batch(ebatches[4])
            emit_strip(35, 67)         # composites for slots [35,67)
            emit_D_batch(dbatches[0])
            emit_D_batch(dbatches[1])
            emit_D_batch(dbatches[2])
            emit_D_batch(dbatches[3])

            nc.sync.dma_start(d_psums[:], acc[:])

    nc.compile()
    return nc


_CACHE = {}
TRACE = False
_LAST = {"exec_time_ns": None, "results": None}


def _get_program():
    if "nc" not in _CACHE:
        _CACHE["nc"] = _build_program()
    return _CACHE["nc"]


def last_exec_time_ns():
    return _LAST["exec_time_ns"]


def kernel(output, target):
    output = np.asarray(output, dtype=np.float32)
    target = np.asarray(target, dtype=np.float32)
    nc = _get_program()

    dif = output[:, 1] - output[:, 0]                  # [B, D, H, W]
    vfull = (dif > 0).astype(np.float32) + 7.0 * target[:, 0] + 1.0
    vpad = np.pad(vfull, ((0, 0), (HALO, HALO), (0, 0), (0, 0)),
                  mode="edge")
    vp = np.zeros(vpad.shape[:3] + (WP,), np.float32)
    vp[..., 2:130] = vpad
    vp[..., 1] = vpad[..., 0]
    vp[..., 130] = vpad[..., 127]
    vp = vp.astype(ml_dtypes.float8_e4m3)
    dif16 = dif.astype(ml_dtypes.bfloat16)
    tgt16 = target[:, 0].astype(ml_dtypes.bfloat16)

    mats = _mats_all().astype(ml_dtypes.float8_e4m3)
    in_maps = []
    for c in range(8):
        s, h = c // 2, c % 2
        d0 = 0 if h == 0 else OWN
        vsl = np.ascontiguousarray(
            vp[s][d0:d0 + DEXT].transpose(1, 0, 2)).reshape(P, DEXT * WP)
        dsl = np.ascontiguousarray(
            dif16[s][d0:d0 + OWN].transpose(1, 0, 2)).reshape(P, OWN * W)
        tsl = np.ascontiguousarray(
            tgt16[s][d0:d0 + OWN].transpose(1, 0, 2)).reshape(P, OWN * W)
        in_maps.append({"dif": dsl, "vst": vsl, "tgt": tsl, "mats": mats})

    res = run_bass_kernel_spmd(nc, in_maps, list(range(8)), trace=TRACE)
    _LAST["exec_time_ns"] = res.exec_time_ns
    _LAST["results"] = res
    tmask = target[:, 0] > 0.5
    pmask = dif > 0
    nonempty = np.zeros(B, bool)
    for s in range(B):
        for msk in (tmask[s], pmask[s]):
            for ax in range(3):
                if nonempty[s]:
                    break
                nonempty[s] |= bool(np.any(np.diff(msk, axis=ax)))
    parts = np.zeros((B, 2), np.float64)   # [S1 = sum pt*m, S2 = sum ps*m]
    for c in range(8):
        ps = res.results[c]["psums"].astype(np.float64)  # [128, 32]
        parts[c // 2, 0] += ps[:, 0::2].sum()
        parts[c // 2, 1] += ps[:, 1::2].sum()
    s_ptm, s_card = parts.T
    dice = (2.0 * s_ptm + EPS) / (s_card + EPS)
    per_sample = np.where(nonempty, 1.0 - dice, 0.0)
    return np.float32(per_sample.sum() / B)
